# revision 5
# baseline (speedup 1.0000x reference)
"""DeltaNet attention TRN2 kernel (nn_DeltaNetAttention_5299989643476).

Strategy: data-parallel over batch (8 batches -> 8 NeuronCores). The
cross-batch cumulative_state scan is tiny ([H, Dh]) and is computed on the
host via an algebraic shortcut, then passed to every core as a small
constant tensor, so the device program needs no collectives.

All four projections run as fp8e4m3 DoubleRow matmuls (2x PE throughput,
half the weight DMA). The input projections are "flipped": the fp8
activation tile (feature-major, already in SBUF) is the stationary operand
and the weight panel streams as the 512-wide moving operand, so the matmul
dominates LDWEIGHTS. That produces token-major K/V/Q; the feature-major
copies (KT/VT/QT) are made by the DMA xbar transpose (dma_start_transpose)
into contiguous per-seq-block tiles -- zero PE/DVE/ACT cost. V is consumed
token-major directly by the num matmuls. phi(x)=elu(x)+1 is computed as
relu(x) + min(exp(x), 1) split across ScalarE (exp), GpSimd (relu) and
DVE (fused min+add via STT). Weights are host-scaled by a power of two
into fp8 range; the inverse scale folds into the PSUM->SBUF copy. attnT is
written as fp8 so the output projection also runs DoubleRow; it iterates
seq-block-outer so the LayerNorm + output DMA of block 0 overlaps the
matmuls of block 1.
"""

import numpy as np
import ml_dtypes

import concourse.bass as bass
import concourse.mybir as mybir
import concourse.tile as tile
from concourse import bacc
from concourse.bass_utils import run_bass_kernel_spmd


def _ensure_axon_hooks():
    """This image's `antenv` lacks `axon_hooks`; if the caller's environment
    sets BASS_TRACE, run_bass_kernel_spmd would crash importing it. Register
    a no-op shim (only when absent) so tracing degrades gracefully."""
    try:
        import antenv.axon_hooks  # noqa: F401
    except ImportError:
        import sys
        import types

        import antenv

        mod = types.ModuleType("antenv.axon_hooks")
        _h = [None]
        mod.set_axon_ntff_profile_hook = lambda h: _h.__setitem__(0, h)
        mod.get_axon_ntff_profile_hook = lambda: _h[0]
        sys.modules["antenv.axon_hooks"] = mod
        antenv.axon_hooks = mod


_ensure_axon_hooks()

B, L, D, H = 8, 256, 2048, 8
DH = D // H            # 256
NB = D // 128          # 16 feature blocks of 128
LB = L // 128          # 2 sequence blocks of 128
EPS = 1e-5

F32 = mybir.dt.float32
BF16 = mybir.dt.bfloat16
F8 = mybir.dt.float8e4
AF = mybir.ActivationFunctionType
OP = mybir.AluOpType
DR = mybir.MatmulPerfMode.DoubleRow

_cache = {}


def _build(scales, plain_ln: bool = False):
    nc = bacc.Bacc(
        "TRN2",
        target_bir_lowering=False,
        debug=False,
        enable_asserts=False,
        num_devices=B,
    )

    qT_d = nc.dram_tensor("qT", [D, L], F8, kind="ExternalInput")
    kT_d = nc.dram_tensor("kT", [D, L], F8, kind="ExternalInput")
    vT_d = nc.dram_tensor("vT", [D, L], F8, kind="ExternalInput")
    qres_d = nc.dram_tensor("qres", [L, D], BF16, kind="ExternalInput")
    wqT_d = nc.dram_tensor("wqT", [D, D], F8, kind="ExternalInput")
    wkT_d = nc.dram_tensor("wkT", [D, D], F8, kind="ExternalInput")
    wvT_d = nc.dram_tensor("wvT", [D, D], F8, kind="ExternalInput")
    woT_d = nc.dram_tensor("woT", [D, D], F8, kind="ExternalInput")
    csp_d = nc.dram_tensor("csp", [128, H * 2], F32, kind="ExternalInput")
    maskT_d = nc.dram_tensor("maskT", [L, L], F8, kind="ExternalInput")
    lng_d = nc.dram_tensor("lng", [D], F32, kind="ExternalInput")
    lnb_d = nc.dram_tensor("lnb", [D], F32, kind="ExternalInput")
    out_d = nc.dram_tensor("out", [L, D], F32, kind="ExternalOutput")

    with tile.TileContext(nc) as tc:
        _body(
            tc, scales,
            qT_d, kT_d, vT_d, qres_d,
            wqT_d, wkT_d, wvT_d, woT_d,
            csp_d, maskT_d, lng_d, lnb_d, out_d,
            plain_ln,
        )
    nc.compile()
    return nc


def _body(tc, scales, qT_d, kT_d, vT_d, qres_d, wqT_d, wkT_d, wvT_d, woT_d,
          csp_d, maskT_d, lng_d, lnb_d, out_d, plain_ln):
    nc = tc.nc
    inv_s = {k: float(1.0 / v) for k, v in scales.items()}

    with (
        tc.tile_pool(name="singles", bufs=1) as singles,
        tc.tile_pool(name="wpool", bufs=4) as wpool,
        tc.tile_pool(name="big", bufs=1) as big,
        tc.tile_pool(name="hgrp", bufs=2) as hgrp,
        tc.tile_pool(name="small", bufs=3) as small,
        tc.tile_pool(name="psA", bufs=2, space="PSUM") as psA,
        tc.tile_pool(name="an_ps", bufs=2, space="PSUM") as an_ps,
        tc.tile_pool(name="dv_ps", bufs=2, space="PSUM") as dv_ps,
    ):
        # ---- input activations (feature-major fp8, used as DR stationary) ----
        xT_in = {}
        for name, dram in (("k", kT_d), ("v", vT_d), ("q", qT_d)):
            t = big.tile([128, NB, L], F8, tag=f"{name}T_in", name=f"{name}T_in")
            xT_in[name] = (t, dram)

        def load_xT(name, h2_eng=None):
            t, dram = xT_in[name]
            r = dram.rearrange("(n p) l -> p n l", p=128)
            nc.gpsimd.dma_start(out=t[:, 0:8, :], in_=r[:, 0:8, :])
            (h2_eng or nc.gpsimd).dma_start(out=t[:, 8:16, :], in_=r[:, 8:16, :])

        # critical path first: kT + the K weight panels
        load_xT("k", h2_eng=nc.scalar)

        w_rs = {
            "k": wkT_d.rearrange("(n p) i -> p n i", p=128),
            "v": wvT_d.rearrange("(n p) i -> p n i", p=128),
            "q": wqT_d.rearrange("(n p) i -> p n i", p=128),
            "o": woT_d.rearrange("(n p) i -> p n i", p=128),
        }

        def panel_dmas(name):
            ts = []
            for ic in range(4):
                w_t = wpool.tile([128, NB, 512], F8, tag="w",
                                 name=f"w_{name}{ic}")
                eng = nc.sync if ic % 2 == 0 else nc.scalar
                eng.dma_start(out=w_t,
                              in_=w_rs[name][:, :, ic * 512:(ic + 1) * 512])
                ts.append(w_t)
            return ts

        wk_ts = panel_dmas("k")

        ones_t = singles.tile([128, 128], BF16)
        nc.vector.memset(ones_t, 1.0)
        eps_t = singles.tile([128, 1], F32)
        nc.vector.memset(eps_t, EPS)

        # dummy matmuls while the first panels stream: keeps the PE-HAM
        # activity monitor busy so the real stream starts at 2.4 GHz
        warm_ps = dv_ps.tile([128, 256], F32, tag="dv", name="warm_ps")
        for _ in range(24):
            nc.tensor.matmul(warm_ps[:, 0:128], ones_t, ones_t,
                             start=True, stop=True)

        csp_t = singles.tile([128, H * 2], F32)
        nc.gpsimd.dma_start(out=csp_t, in_=csp_d.ap())

        # ---- flipped projections: X[l, i] = sum_j xT[j, l] * wT[j, i] ----
        K_tok = big.tile([128, LB, D], BF16, tag="ktok", name="ktok")
        V_t = big.tile([128, LB, D], BF16, tag="V", name="V")
        Q_tok = big.tile([128, LB, D], BF16, tag="qtok", name="qtok")

        ccount = [0]

        def flip_proj(name, tok_t, w_ts):
            x_t = xT_in[name][0]
            for ic in range(4):
                w_t = w_ts[ic]
                isl = slice(ic * 512, (ic + 1) * 512)
                ps = psA.tile([128, 2, 512], F32, tag="pk")
                for lb in range(LB):
                    for jj in range(8):
                        nc.tensor.matmul(
                            ps[:, lb, :],
                            x_t[:, 2 * jj:2 * jj + 2, lb * 128:(lb + 1) * 128],
                            w_t[:, 2 * jj:2 * jj + 2, :],
                            start=(jj == 0),
                            stop=(jj == 7),
                            perf_mode=DR,
                        )
                # PSUM -> SBUF with the weight-scale fold, alternating DVE/ACT
                if ccount[0] % 2 == 0:
                    nc.vector.tensor_scalar(
                        out=tok_t[:, :, isl], in0=ps,
                        scalar1=inv_s[name], scalar2=None, op0=OP.mult,
                    )
                else:
                    nc.scalar.activation(
                        out=tok_t[:, :, isl], in_=ps, func=AF.Identity,
                        scale=inv_s[name],
                    )
                ccount[0] += 1

        # per-lb feature-major tiles via DMA xbar transpose (contiguous dst)
        def xbar_T(tok_t, nm):
            outs = []
            for lb in range(LB):
                t = big.tile([128, NB, 128], BF16, tag=f"{nm}T{lb}",
                             name=f"{nm}T{lb}")
                eng = nc.sync if lb == 0 else nc.scalar
                eng.dma_start_transpose(out=t, in_=tok_t[:, lb, :])
                outs.append(t)
            return outs

        flip_proj("k", K_tok, wk_ts)
        wv_ts = panel_dmas("v")
        load_xT("v")
        KT = xbar_T(K_tok, "k")

        flip_proj("v", V_t, wv_ts)
        wq_ts = panel_dmas("q")
        load_xT("q")
        VT = xbar_T(V_t, "v")

        # wo preloaded whole (used ~25us later; single big DMA streams behind)
        wo_t = singles.tile([128, NB, D], F8, name="wo")
        nc.sync.dma_start(out=wo_t, in_=w_rs["o"])

        maskT_t = singles.tile([128, LB, L], F8)
        nc.gpsimd.dma_start(out=maskT_t,
                            in_=maskT_d.rearrange("(a p) l -> p a l", p=128))
        qres_t = []
        for lb in range(LB):
            t = big.tile([128, D], BF16, tag=f"qres{lb}", name=f"qres{lb}")
            nc.gpsimd.dma_start(out=t, in_=qres_d.ap()[lb * 128:(lb + 1) * 128, :])
            qres_t.append(t)
        lng_t = lnb_t = None
        if not plain_ln:
            lng_t = singles.tile([128, D], F32)
            nc.gpsimd.dma_start(out=lng_t,
                                in_=lng_d.ap().partition_broadcast(128))
            lnb_t = singles.tile([128, D], F32)
            nc.gpsimd.dma_start(out=lnb_t,
                                in_=lnb_d.ap().partition_broadcast(128))

        flip_proj("q", Q_tok, wq_ts)
        QT = xbar_T(Q_tok, "q")

        # ---- attention: per head-group kv + q-mod + phi ----
        pk_t = big.tile([128, NB, L], BF16, tag="pk_t", name="pk_t")
        pq_t = big.tile([128, NB, L], BF16, tag="pq_t", name="pq_t")
        attnT_t = big.tile([128, NB, L], F8, tag="attnT", name="attnT")

        for g in range(4):
            gsl = slice(4 * g, 4 * g + 4)
            # pk = phi(KT) = relu(KT) + min(exp(KT), 1), per lb block
            for lb in range(LB):
                ex = hgrp.tile([128, 4, 128], BF16, tag="eq")
                rx = hgrp.tile([128, 4, 128], BF16, tag="rx")
                nc.scalar.activation(ex, KT[lb][:, gsl, :], AF.Exp)
                nc.gpsimd.tensor_scalar_max(rx, KT[lb][:, gsl, :], 0.0)
                nc.vector.scalar_tensor_tensor(
                    out=pk_t[:, gsl, lb * 128:(lb + 1) * 128],
                    in0=ex, scalar=1.0, in1=rx, op0=OP.min, op1=OP.add,
                )
            # kv + q-mod for the group's 2 heads
            kvm = hgrp.tile([128, 4, L], BF16, tag="kvm")
            for hh in range(2):
                h = 2 * g + hh
                n0 = 2 * h
                ps = psA.tile([128, 2, L], F32, tag="pk")
                for mb in range(2):
                    for db in range(2):
                        for lb in range(LB):
                            nc.tensor.matmul(
                                ps[:, mb, lb * 128:(lb + 1) * 128],
                                VT[mb][:, n0 + db, :],
                                KT[lb][:, n0 + db, :],
                                start=(db == 0),
                                stop=(db == 1),
                            )
                for mb in range(2):
                    # q_mod = (alpha*Q) * (kv + cs*(1-alpha)/alpha); alpha is
                    # folded into Wq on the host.
                    for lb in range(LB):
                        nc.vector.scalar_tensor_tensor(
                            out=kvm[:, 2 * hh + mb, lb * 128:(lb + 1) * 128],
                            in0=ps[:, mb, lb * 128:(lb + 1) * 128],
                            scalar=csp_t[:, n0 + mb:n0 + mb + 1],
                            in1=QT[lb][:, n0 + mb, :],
                            op0=OP.add,
                            op1=OP.mult,
                        )
            # pq = phi(kvm)
            eq = hgrp.tile([128, 4, L], BF16, tag="eqw")
            rq = hgrp.tile([128, 4, L], BF16, tag="rqw")
            nc.scalar.activation(eq, kvm, AF.Exp)
            nc.gpsimd.tensor_scalar_max(rq, kvm, 0.0)
            nc.vector.scalar_tensor_tensor(
                out=pq_t[:, gsl, :], in0=eq, scalar=1.0, in1=rq,
                op0=OP.min, op1=OP.add,
            )

        # ---- per-head: A matmul, mask, den, num, attnT ----
        for h in range(H):
            n0 = 2 * h
            # causal block structure of AT[i, l] (i<=l kept):
            #   ib=0: l<128 lower-triangular, l>=128 all-ones
            #   ib=1: l<128 all-zero (skipped entirely), l>=128 triangular
            a_ps = an_ps.tile([128, 2, L], F32, tag="an")
            for db in range(2):
                nc.tensor.matmul(
                    a_ps[:, 0, :],
                    pk_t[:, n0 + db, 0:128],
                    pq_t[:, n0 + db, :],
                    start=(db == 0), stop=(db == 1),
                )
            for db in range(2):
                nc.tensor.matmul(
                    a_ps[:, 1, 128:L],
                    pk_t[:, n0 + db, 128:L],
                    pq_t[:, n0 + db, 128:L],
                    start=(db == 0), stop=(db == 1),
                )
            am = small.tile([128, LB, L], BF16, tag="am")
            nc.vector.tensor_mul(am[:, 0, 0:128], a_ps[:, 0, 0:128],
                                 maskT_t[:, 0, 0:128])
            nc.scalar.copy(out=am[:, 0, 128:L], in_=a_ps[:, 0, 128:L])
            nc.vector.tensor_mul(am[:, 1, 128:L], a_ps[:, 1, 128:L],
                                 maskT_t[:, 1, 128:L])

            den_ps = dv_ps.tile([128, L], F32, tag="dv", name="den_ps")
            nc.tensor.matmul(den_ps[:, 0:128], ones_t, am[:, 0, 0:128],
                             start=True, stop=True)
            nc.tensor.matmul(den_ps[:, 128:L], ones_t, am[:, 0, 128:L],
                             start=True, stop=False)
            nc.tensor.matmul(den_ps[:, 128:L], ones_t, am[:, 1, 128:L],
                             start=False, stop=True)
            # den is a sum of strictly positive phi-products, so the
            # reference's 1e-8 clamp can never bind.
            rden = small.tile([128, L], F32, tag="rden")
            nc.vector.reciprocal_approx_fast(out=rden, in_=den_ps)

            n_ps = an_ps.tile([128, 2, L], F32, tag="an")
            for db in range(2):
                v0 = V_t[:, 0, h * DH + db * 128:h * DH + (db + 1) * 128]
                v1 = V_t[:, 1, h * DH + db * 128:h * DH + (db + 1) * 128]
                nc.tensor.matmul(n_ps[:, db, 0:128], v0, am[:, 0, 0:128],
                                 start=True, stop=True)
                nc.tensor.matmul(n_ps[:, db, 128:L], v0, am[:, 0, 128:L],
                                 start=True, stop=False)
                nc.tensor.matmul(n_ps[:, db, 128:L], v1, am[:, 1, 128:L],
                                 start=False, stop=True)
            for db in range(2):
                nc.vector.tensor_mul(attnT_t[:, n0 + db, :], n_ps[:, db, :], rden)

        # Trigger the sqrt ACT-table load now — after ScalarE's last
        # Copy/Exp user, off the LN tail's critical path.
        warm_sqrt = singles.tile([128, 1], F32)
        nc.scalar.activation(warm_sqrt, eps_t, AF.Sqrt)

        # ---- output projection (fp8 DR, stationary=attnT) + residual + LN ----
        # seq-block-outer so lb=0's LN + output DMA overlap lb=1's matmuls
        x_sb = [big.tile([128, D], F32, tag=f"x{lb}", name=f"x{lb}")
                for lb in range(LB)]
        stats = [small.tile([128, 4, 6], F32, tag=f"stats{lb}",
                            name=f"stats{lb}", bufs=1) for lb in range(LB)]
        for lb in range(LB):
            for nq in range(4):
                psf = psA.tile([128, 512], F32, tag="pk")
                for jj in range(8):
                    nc.tensor.matmul(
                        psf,
                        attnT_t[:, 2 * jj:2 * jj + 2, lb * 128:(lb + 1) * 128],
                        wo_t[:, 2 * jj:2 * jj + 2, nq * 512:(nq + 1) * 512],
                        start=(jj == 0),
                        stop=(jj == 7),
                        perf_mode=DR,
                    )
                # x = o/s_o + (query + bo)
                sl = slice(nq * 512, (nq + 1) * 512)
                nc.vector.scalar_tensor_tensor(
                    out=x_sb[lb][:, sl], in0=psf, scalar=inv_s["o"],
                    in1=qres_t[lb][:, sl], op0=OP.mult, op1=OP.add,
                )
                nc.vector.bn_stats(out=stats[lb][:, nq, :],
                                   in_=x_sb[lb][:, sl])

            x = x_sb[lb]
            mv = small.tile([128, 2], F32, tag="mv")
            nc.vector.bn_aggr(out=mv, in_=stats[lb])
            sd = small.tile([128, 1], F32, tag="sd")
            nc.scalar.activation(sd, mv[:, 1:2], AF.Sqrt, bias=eps_t)
            nc.vector.reciprocal_approx_fast(out=sd, in_=sd)
            nsdmu = small.tile([128, 1], F32, tag="nsdmu")
            nc.vector.tensor_scalar(
                out=nsdmu, in0=sd, scalar1=mv[:, 0:1], scalar2=-1.0,
                op0=OP.mult, op1=OP.mult,
            )
            for ch in range(4):  # quarters, so DVE work overlaps output DMA
                sl = slice(ch * (D // 4), (ch + 1) * (D // 4))
                if plain_ln:
                    # ln_g == 1, ln_b == 0: fused (x - mu) * rstd, split
                    # across DVE and the idle ScalarE (as rstd*x - rstd*mu)
                    if ch % 2 == 0:
                        nc.vector.tensor_scalar(
                            out=x[:, sl], in0=x[:, sl], scalar1=mv[:, 0:1],
                            scalar2=sd, op0=OP.subtract, op1=OP.mult,
                        )
                    else:
                        nc.scalar.activation(
                            out=x[:, sl], in_=x[:, sl], func=AF.Identity,
                            bias=nsdmu, scale=sd,
                        )
                else:
                    nc.vector.tensor_scalar(
                        out=x[:, sl], in0=x[:, sl], scalar1=mv[:, 0:1],
                        scalar2=None, op0=OP.subtract,
                    )
                    nc.vector.scalar_tensor_tensor(
                        out=x[:, sl], in0=x[:, sl], scalar=sd, in1=lng_t[:, sl],
                        op0=OP.mult, op1=OP.mult,
                    )
                    nc.vector.tensor_add(x[:, sl], x[:, sl], lnb_t[:, sl])
                oeng = nc.sync if ch % 2 == 0 else nc.gpsimd
                oeng.dma_start(
                    out=out_d.ap()[lb * 128:(lb + 1) * 128, sl], in_=x[:, sl])


def _pow2scale(w, target=120.0):
    m = float(np.abs(w).max())
    if m == 0.0:
        return 1.0
    return float(2.0 ** np.floor(np.log2(target / m)))


def _f8(x):
    return np.clip(x, -240.0, 240.0).astype(ml_dtypes.float8_e4m3)


def _host_prep(query, key, value, Wq, Wk, Wv, Wo, bo, ln_g, ln_b, alpha, beta):
    """Host-side: cumulative_state shortcut + fp8 layout/dtype marshaling."""
    a, b = float(alpha), float(beta)
    f64 = np.float64
    # mean over (batch, l) of kv[b,h,l,m] = (1/(B*L)) sum_b Ksum[b,h,:].V[b,h,m,:]
    keysum = key.astype(f64).sum(axis=1)                      # [B, D]
    Ksum = (keysum @ Wk.T.astype(f64)).reshape(B, H, DH)      # [B, H, DH]
    WvH = Wv.astype(f64).reshape(H, DH, D)
    wv_eff = np.einsum("hdj,bhd->bhj", WvH, Ksum, optimize=True)      # [B,H,D]
    contrib = np.einsum("bmj,bhj->hm", value.astype(f64), wv_eff, optimize=True)
    mean_kv = contrib / (B * L)                               # [H, DH]
    cs = np.zeros((H, DH), f64)
    c = np.zeros(DH, f64)
    for h in range(H):
        cs[h] = c
        c = b * c + a * mean_kv[h]
    # q_mod = Q*((1-a)*cs + a*kv) = (a*Q)*(kv + (1-a)/a*cs); a is folded
    # into Wq below, and this is cs*(1-a)/a:
    csp = ((1.0 - a) / a * cs if a != 0 else 0.0 * cs).astype(np.float32)
    csp_dev = np.ascontiguousarray(
        csp.reshape(H, 2, 128).transpose(2, 0, 1).reshape(128, H * 2)
    )
    plain_ln = bool(np.all(ln_g == 1.0) and np.all(ln_b == 0.0))

    bf = ml_dtypes.bfloat16
    wq_eff = a * Wq.T
    wk_eff, wv_eff_m, wo_eff = Wk.T, Wv.T, Wo.T
    scales = {
        "q": _pow2scale(wq_eff), "k": _pow2scale(wk_eff),
        "v": _pow2scale(wv_eff_m), "o": _pow2scale(wo_eff),
    }

    qT = _f8(np.ascontiguousarray(query.transpose(0, 2, 1)))
    kT = _f8(np.ascontiguousarray(key.transpose(0, 2, 1)))
    vT = _f8(np.ascontiguousarray(value.transpose(0, 2, 1)))
    wqT = _f8(np.ascontiguousarray(wq_eff * scales["q"]))
    wkT = _f8(np.ascontiguousarray(wk_eff * scales["k"]))
    wvT = _f8(np.ascontiguousarray(wv_eff_m * scales["v"]))
    woT = _f8(np.ascontiguousarray(wo_eff * scales["o"]))
    qres = (query + bo[None, None, :]).astype(bf)
    maskT = _f8(np.triu(np.ones((L, L), np.float32)))  # maskT[i,l]=1 iff i<=l

    in_maps = []
    for c_ in range(B):
        in_maps.append({
            "qT": qT[c_], "kT": kT[c_], "vT": vT[c_],
            "qres": qres[c_],
            "wqT": wqT, "wkT": wkT, "wvT": wvT, "woT": woT,
            "csp": csp_dev, "maskT": maskT,
            "lng": ln_g.astype(np.float32), "lnb": ln_b.astype(np.float32),
        })
    return in_maps, scales, plain_ln


def get_nc(scales, plain_ln: bool = True):
    key = (tuple(sorted(scales.items())), bool(plain_ln))
    if key not in _cache:
        _cache[key] = _build(scales, bool(plain_ln))
    return _cache[key]


def kernel(query, key, value, Wq, Wk, Wv, Wo, bo, ln_g, ln_b, alpha, beta,
           _trace=False, _trace_kwargs=None):
    args = [np.asarray(x) for x in
            (query, key, value, Wq, Wk, Wv, Wo, bo, ln_g, ln_b, alpha, beta)]
    in_maps, scales, plain_ln = _host_prep(*args)
    nc = get_nc(scales, plain_ln)
    res = run_bass_kernel_spmd(
        nc, in_maps, core_ids=list(range(B)),
        trace=_trace, **(_trace_kwargs or {}),
    )
    out = np.stack([res.results[c]["out"] for c in range(B)], axis=0)
    if _trace:
        kernel._last_results = res
    return out


# revision 7
# speedup vs baseline: 1.4534x; 1.4534x over previous
"""DeltaNet attention TRN2 kernel (nn_DeltaNetAttention_5299989643476).

Strategy: data-parallel over batch (8 batches -> 8 NeuronCores). The
cross-batch cumulative_state scan is tiny ([H, Dh]) and is computed on the
host via an algebraic shortcut, then passed to every core as a small
constant tensor, so the device program needs no collectives.

All four projections run as fp8e4m3 DoubleRow matmuls (2x PE throughput,
half the weight DMA). The input projections are "flipped": the fp8
activation tile (feature-major, already in SBUF) is the stationary operand
and the weight panel streams as the 512-wide moving operand, so the matmul
dominates LDWEIGHTS. That produces token-major K/V/Q; the feature-major
copies (KT/VT/QT) are made by the DMA xbar transpose (dma_start_transpose)
into contiguous per-seq-block tiles -- zero PE/DVE/ACT cost. V is consumed
token-major directly by the num matmuls. phi(x)=elu(x)+1 is computed as
relu(x) + min(exp(x), 1) split across ScalarE (exp), GpSimd (relu) and
DVE (fused min+add via STT). Weights are host-scaled by a power of two
into fp8 range; the inverse scale folds into the PSUM->SBUF copy. attnT is
written as fp8 so the output projection also runs DoubleRow; it iterates
seq-block-outer so the LayerNorm + output DMA of block 0 overlaps the
matmuls of block 1.
"""

import numpy as np
import ml_dtypes

import concourse.bass as bass
import concourse.mybir as mybir
import concourse.tile as tile
from concourse import bacc
from concourse.bass_utils import run_bass_kernel_spmd


def _ensure_axon_hooks():
    """This image's `antenv` lacks `axon_hooks`; if the caller's environment
    sets BASS_TRACE, run_bass_kernel_spmd would crash importing it. Register
    a no-op shim (only when absent) so tracing degrades gracefully."""
    try:
        import antenv.axon_hooks  # noqa: F401
    except ImportError:
        import sys
        import types

        import antenv

        mod = types.ModuleType("antenv.axon_hooks")
        _h = [None]
        mod.set_axon_ntff_profile_hook = lambda h: _h.__setitem__(0, h)
        mod.get_axon_ntff_profile_hook = lambda: _h[0]
        sys.modules["antenv.axon_hooks"] = mod
        antenv.axon_hooks = mod


_ensure_axon_hooks()

B, L, D, H = 8, 256, 2048, 8
DH = D // H            # 256
NB = D // 128          # 16 feature blocks of 128
LB = L // 128          # 2 sequence blocks of 128
EPS = 1e-5

F32 = mybir.dt.float32
BF16 = mybir.dt.bfloat16
F8 = mybir.dt.float8e4
AF = mybir.ActivationFunctionType
OP = mybir.AluOpType
DR = mybir.MatmulPerfMode.DoubleRow

_cache = {}


def _build(scales, plain_ln: bool = False):
    nc = bacc.Bacc(
        "TRN2",
        target_bir_lowering=False,
        debug=False,
        enable_asserts=False,
        num_devices=B,
    )

    qT_d = nc.dram_tensor("qT", [D, L], F8, kind="ExternalInput")
    kT_d = nc.dram_tensor("kT", [D, L], F8, kind="ExternalInput")
    vT_d = nc.dram_tensor("vT", [D, L], F8, kind="ExternalInput")
    qres_d = nc.dram_tensor("qres", [L, D], BF16, kind="ExternalInput")
    wqT_d = nc.dram_tensor("wqT", [D, D], F8, kind="ExternalInput")
    wkT_d = nc.dram_tensor("wkT", [D, D], F8, kind="ExternalInput")
    wvT_d = nc.dram_tensor("wvT", [D, D], F8, kind="ExternalInput")
    woT_d = nc.dram_tensor("woT", [D, D], F8, kind="ExternalInput")
    csp_d = nc.dram_tensor("csp", [128, H * 2], F32, kind="ExternalInput")
    maskT_d = nc.dram_tensor("maskT", [L, L], F8, kind="ExternalInput")
    lng_d = nc.dram_tensor("lng", [D], F32, kind="ExternalInput")
    lnb_d = nc.dram_tensor("lnb", [D], F32, kind="ExternalInput")
    out_d = nc.dram_tensor("out", [L, D], F32, kind="ExternalOutput")

    with tile.TileContext(nc) as tc:
        _body(
            tc, scales,
            qT_d, kT_d, vT_d, qres_d,
            wqT_d, wkT_d, wvT_d, woT_d,
            csp_d, maskT_d, lng_d, lnb_d, out_d,
            plain_ln,
        )
    nc.compile()
    return nc


def _body(tc, scales, qT_d, kT_d, vT_d, qres_d, wqT_d, wkT_d, wvT_d, woT_d,
          csp_d, maskT_d, lng_d, lnb_d, out_d, plain_ln):
    nc = tc.nc
    inv_s = {k: float(1.0 / v) for k, v in scales.items()}

    with (
        tc.tile_pool(name="singles", bufs=1) as singles,
        tc.tile_pool(name="wpool", bufs=4) as wpool,
        tc.tile_pool(name="big", bufs=1) as big,
        tc.tile_pool(name="hgrp", bufs=2) as hgrp,
        tc.tile_pool(name="small", bufs=3) as small,
        tc.tile_pool(name="psA", bufs=2, space="PSUM") as psA,
        tc.tile_pool(name="an_ps", bufs=2, space="PSUM") as an_ps,
        tc.tile_pool(name="dv_ps", bufs=2, space="PSUM") as dv_ps,
    ):
        # ---- input activations (feature-major fp8, used as DR stationary) ----
        xT_in = {}
        for name, dram in (("k", kT_d), ("v", vT_d), ("q", qT_d)):
            t = big.tile([128, NB, L], F8, tag=f"{name}T_in", name=f"{name}T_in")
            xT_in[name] = (t, dram)

        def load_xT(name, h2_eng=None):
            t, dram = xT_in[name]
            r = dram.rearrange("(n p) l -> p n l", p=128)
            nc.gpsimd.dma_start(out=t[:, 0:8, :], in_=r[:, 0:8, :])
            (h2_eng or nc.gpsimd).dma_start(out=t[:, 8:16, :], in_=r[:, 8:16, :])

        # critical path first: kT + the K weight panels
        load_xT("k", h2_eng=nc.scalar)

        w_rs = {
            "k": wkT_d.rearrange("(n p) i -> p n i", p=128),
            "v": wvT_d.rearrange("(n p) i -> p n i", p=128),
            "q": wqT_d.rearrange("(n p) i -> p n i", p=128),
            "o": woT_d.rearrange("(n p) i -> p n i", p=128),
        }

        def panel_dmas(name):
            ts = []
            for ic in range(4):
                w_t = wpool.tile([128, NB, 512], F8, tag="w",
                                 name=f"w_{name}{ic}")
                eng = nc.sync if ic % 2 == 0 else nc.scalar
                eng.dma_start(out=w_t,
                              in_=w_rs[name][:, :, ic * 512:(ic + 1) * 512])
                ts.append(w_t)
            return ts

        wk_ts = panel_dmas("k")

        ones_t = singles.tile([128, 128], BF16)
        nc.vector.memset(ones_t, 1.0)
        eps_t = singles.tile([128, 1], F32)
        nc.vector.memset(eps_t, EPS)

        # dummy matmuls while the first panels stream: keeps the PE-HAM
        # activity monitor busy so the real stream starts at 2.4 GHz
        warm_ps = dv_ps.tile([128, 256], F32, tag="dv", name="warm_ps")
        for _ in range(24):
            nc.tensor.matmul(warm_ps[:, 0:128], ones_t, ones_t,
                             start=True, stop=True)

        csp_t = singles.tile([128, H * 2], F32)
        nc.gpsimd.dma_start(out=csp_t, in_=csp_d.ap())

        # ---- flipped projections: X[l, i] = sum_j xT[j, l] * wT[j, i] ----
        K_tok = big.tile([128, LB, D], BF16, tag="ktok", name="ktok")
        V_t = big.tile([128, LB, D], BF16, tag="V", name="V")
        Q_tok = big.tile([128, LB, D], BF16, tag="qtok", name="qtok")

        ccount = [0]

        def flip_proj(name, tok_t, w_ts):
            x_t = xT_in[name][0]
            for ic in range(4):
                w_t = w_ts[ic]
                isl = slice(ic * 512, (ic + 1) * 512)
                ps = psA.tile([128, 2, 512], F32, tag="pk")
                for lb in range(LB):
                    for jj in range(8):
                        nc.tensor.matmul(
                            ps[:, lb, :],
                            x_t[:, 2 * jj:2 * jj + 2, lb * 128:(lb + 1) * 128],
                            w_t[:, 2 * jj:2 * jj + 2, :],
                            start=(jj == 0),
                            stop=(jj == 7),
                            perf_mode=DR,
                        )
                # PSUM -> SBUF with the weight-scale fold, alternating DVE/ACT
                if ccount[0] % 2 == 0:
                    nc.vector.tensor_scalar(
                        out=tok_t[:, :, isl], in0=ps,
                        scalar1=inv_s[name], scalar2=None, op0=OP.mult,
                    )
                else:
                    nc.scalar.activation(
                        out=tok_t[:, :, isl], in_=ps, func=AF.Identity,
                        scale=inv_s[name],
                    )
                ccount[0] += 1

        # feature-major [128, lb, n, l2] tiles via DMA xbar transpose; each
        # lb slice is a contiguous destination (required for correctness)
        def xbar_T(tok_t, nm):
            t = big.tile([128, LB, NB, 128], BF16, tag=f"{nm}T2",
                         name=f"{nm}T2")
            for lb in range(LB):
                eng = nc.sync if lb == 0 else nc.scalar
                eng.dma_start_transpose(out=t[:, lb], in_=tok_t[:, lb, :])
            return t

        flip_proj("k", K_tok, wk_ts)
        wv_ts = panel_dmas("v")
        load_xT("v")
        KT = xbar_T(K_tok, "k")

        flip_proj("v", V_t, wv_ts)
        wq_ts = panel_dmas("q")
        load_xT("q")
        VT = xbar_T(V_t, "v")

        # wo preloaded whole (used ~25us later; the big DMA streams behind
        # the q panels)
        wo_t = singles.tile([128, NB, D], F8, name="wo")
        nc.sync.dma_start(out=wo_t, in_=w_rs["o"])

        maskT_t = singles.tile([128, LB, L], F8)
        nc.gpsimd.dma_start(out=maskT_t,
                            in_=maskT_d.rearrange("(a p) l -> p a l", p=128))
        qres_t = []
        for lb in range(LB):
            t = big.tile([128, D], BF16, tag=f"qres{lb}", name=f"qres{lb}")
            nc.gpsimd.dma_start(out=t, in_=qres_d.ap()[lb * 128:(lb + 1) * 128, :])
            qres_t.append(t)
        lng_t = lnb_t = None
        if not plain_ln:
            lng_t = singles.tile([128, D], F32)
            nc.gpsimd.dma_start(out=lng_t,
                                in_=lng_d.ap().partition_broadcast(128))
            lnb_t = singles.tile([128, D], F32)
            nc.gpsimd.dma_start(out=lnb_t,
                                in_=lnb_d.ap().partition_broadcast(128))

        flip_proj("q", Q_tok, wq_ts)
        QT = xbar_T(Q_tok, "q")

        # ---- attention: per head-group kv + q-mod + phi ----
        pk_t = big.tile([128, NB, L], BF16, tag="pk_t", name="pk_t")
        pq_t = big.tile([128, NB, L], BF16, tag="pq_t", name="pq_t")
        attnT_t = big.tile([128, NB, L], F8, tag="attnT", name="attnT")

        for g in range(4):
            gsl = slice(4 * g, 4 * g + 4)
            # pk = phi(KT) = relu(KT) + min(exp(KT), 1); exp+relu on ScalarE,
            # fused min+add on DVE
            for lb in range(LB):
                ex = hgrp.tile([128, 4, 128], BF16, tag="eq")
                rx = hgrp.tile([128, 4, 128], BF16, tag="rx")
                nc.scalar.activation(ex, KT[:, lb, gsl, :], AF.Exp)
                nc.scalar.activation(rx, KT[:, lb, gsl, :], AF.Relu)
                nc.vector.scalar_tensor_tensor(
                    out=pk_t[:, gsl, lb * 128:(lb + 1) * 128],
                    in0=ex, scalar=1.0, in1=rx, op0=OP.min, op1=OP.add,
                )
            # kv + q-mod for the group's 2 heads
            kvm = hgrp.tile([128, 4, L], BF16, tag="kvm")
            for hh in range(2):
                h = 2 * g + hh
                n0 = 2 * h
                ps = psA.tile([128, 2, L], F32, tag="pk")
                for mb in range(2):
                    for db in range(2):
                        for lb in range(LB):
                            nc.tensor.matmul(
                                ps[:, mb, lb * 128:(lb + 1) * 128],
                                VT[:, mb, n0 + db, :],
                                KT[:, lb, n0 + db, :],
                                start=(db == 0),
                                stop=(db == 1),
                            )
                for mb in range(2):
                    # q_mod = (alpha*Q) * (kv + cs*(1-alpha)/alpha); alpha is
                    # folded into Wq on the host.
                    nc.vector.scalar_tensor_tensor(
                        out=kvm[:, 2 * hh + mb, :],
                        in0=ps[:, mb, :],
                        scalar=csp_t[:, n0 + mb:n0 + mb + 1],
                        in1=QT[:, :, n0 + mb, :],
                        op0=OP.add,
                        op1=OP.mult,
                    )
            # pq = phi(kvm)
            eq = hgrp.tile([128, 4, L], BF16, tag="eqw")
            rq = hgrp.tile([128, 4, L], BF16, tag="rqw")
            nc.scalar.activation(eq, kvm, AF.Exp)
            nc.scalar.activation(rq, kvm, AF.Relu)
            nc.vector.scalar_tensor_tensor(
                out=pq_t[:, gsl, :], in0=eq, scalar=1.0, in1=rq,
                op0=OP.min, op1=OP.add,
            )

        # ---- per-head: A matmul, mask, den, num, attnT ----
        for h in range(H):
            n0 = 2 * h
            # causal block structure of AT[i, l] (i<=l kept):
            #   ib=0: l<128 lower-triangular, l>=128 all-ones
            #   ib=1: l<128 all-zero (skipped entirely), l>=128 triangular
            a_ps = an_ps.tile([128, 2, L], F32, tag="an")
            for db in range(2):
                nc.tensor.matmul(
                    a_ps[:, 0, :],
                    pk_t[:, n0 + db, 0:128],
                    pq_t[:, n0 + db, :],
                    start=(db == 0), stop=(db == 1),
                )
            for db in range(2):
                nc.tensor.matmul(
                    a_ps[:, 1, 128:L],
                    pk_t[:, n0 + db, 128:L],
                    pq_t[:, n0 + db, 128:L],
                    start=(db == 0), stop=(db == 1),
                )
            am = small.tile([128, LB, L], BF16, tag="am")
            nc.vector.tensor_mul(am[:, 0, 0:128], a_ps[:, 0, 0:128],
                                 maskT_t[:, 0, 0:128])
            nc.scalar.copy(out=am[:, 0, 128:L], in_=a_ps[:, 0, 128:L])
            nc.vector.tensor_mul(am[:, 1, 128:L], a_ps[:, 1, 128:L],
                                 maskT_t[:, 1, 128:L])

            den_ps = dv_ps.tile([128, L], F32, tag="dv", name="den_ps")
            nc.tensor.matmul(den_ps[:, 0:128], ones_t, am[:, 0, 0:128],
                             start=True, stop=True)
            nc.tensor.matmul(den_ps[:, 128:L], ones_t, am[:, 0, 128:L],
                             start=True, stop=False)
            nc.tensor.matmul(den_ps[:, 128:L], ones_t, am[:, 1, 128:L],
                             start=False, stop=True)
            # den is a sum of strictly positive phi-products, so the
            # reference's 1e-8 clamp can never bind.
            rden = small.tile([128, L], F32, tag="rden")
            nc.vector.reciprocal_approx_fast(out=rden, in_=den_ps)

            n_ps = an_ps.tile([128, 2, L], F32, tag="an")
            for db in range(2):
                v0 = V_t[:, 0, h * DH + db * 128:h * DH + (db + 1) * 128]
                v1 = V_t[:, 1, h * DH + db * 128:h * DH + (db + 1) * 128]
                nc.tensor.matmul(n_ps[:, db, 0:128], v0, am[:, 0, 0:128],
                                 start=True, stop=True)
                nc.tensor.matmul(n_ps[:, db, 128:L], v0, am[:, 0, 128:L],
                                 start=True, stop=False)
                nc.tensor.matmul(n_ps[:, db, 128:L], v1, am[:, 1, 128:L],
                                 start=False, stop=True)
            for db in range(2):
                nc.vector.tensor_mul(attnT_t[:, n0 + db, :], n_ps[:, db, :], rden)

        # Trigger the sqrt ACT-table load now — after ScalarE's last
        # Copy/Exp user, off the LN tail's critical path.
        warm_sqrt = singles.tile([128, 1], F32)
        nc.scalar.activation(warm_sqrt, eps_t, AF.Sqrt)

        # ---- output projection (fp8 DR, stationary=attnT) + residual + LN ----
        # seq-block-outer so lb=0's LN + output DMA overlap lb=1's matmuls
        x_sb = [big.tile([128, D], F32, tag=f"x{lb}", name=f"x{lb}")
                for lb in range(LB)]
        stats = [small.tile([128, 4, 6], F32, tag=f"stats{lb}",
                            name=f"stats{lb}", bufs=1) for lb in range(LB)]
        for lb in range(LB):
            for nq in range(4):
                psf = psA.tile([128, 512], F32, tag="pk")
                for jj in range(8):
                    nc.tensor.matmul(
                        psf,
                        attnT_t[:, 2 * jj:2 * jj + 2, lb * 128:(lb + 1) * 128],
                        wo_t[:, 2 * jj:2 * jj + 2, nq * 512:(nq + 1) * 512],
                        start=(jj == 0),
                        stop=(jj == 7),
                        perf_mode=DR,
                    )
                # x = o/s_o + (query + bo)
                sl = slice(nq * 512, (nq + 1) * 512)
                nc.vector.scalar_tensor_tensor(
                    out=x_sb[lb][:, sl], in0=psf, scalar=inv_s["o"],
                    in1=qres_t[lb][:, sl], op0=OP.mult, op1=OP.add,
                )
                nc.vector.bn_stats(out=stats[lb][:, nq, :],
                                   in_=x_sb[lb][:, sl])

            x = x_sb[lb]
            mv = small.tile([128, 2], F32, tag="mv")
            nc.vector.bn_aggr(out=mv, in_=stats[lb])
            sd = small.tile([128, 1], F32, tag="sd")
            nc.scalar.activation(sd, mv[:, 1:2], AF.Sqrt, bias=eps_t)
            nc.vector.reciprocal_approx_fast(out=sd, in_=sd)
            nsdmu = small.tile([128, 1], F32, tag="nsdmu")
            nc.vector.tensor_scalar(
                out=nsdmu, in0=sd, scalar1=mv[:, 0:1], scalar2=-1.0,
                op0=OP.mult, op1=OP.mult,
            )
            for ch in range(4):  # quarters, so DVE work overlaps output DMA
                sl = slice(ch * (D // 4), (ch + 1) * (D // 4))
                if plain_ln:
                    # ln_g == 1, ln_b == 0: fused (x - mu) * rstd, split
                    # across DVE and the idle ScalarE (as rstd*x - rstd*mu)
                    if ch % 2 == 0:
                        nc.vector.tensor_scalar(
                            out=x[:, sl], in0=x[:, sl], scalar1=mv[:, 0:1],
                            scalar2=sd, op0=OP.subtract, op1=OP.mult,
                        )
                    else:
                        nc.scalar.activation(
                            out=x[:, sl], in_=x[:, sl], func=AF.Identity,
                            bias=nsdmu, scale=sd,
                        )
                else:
                    nc.vector.tensor_scalar(
                        out=x[:, sl], in0=x[:, sl], scalar1=mv[:, 0:1],
                        scalar2=None, op0=OP.subtract,
                    )
                    nc.vector.scalar_tensor_tensor(
                        out=x[:, sl], in0=x[:, sl], scalar=sd, in1=lng_t[:, sl],
                        op0=OP.mult, op1=OP.mult,
                    )
                    nc.vector.tensor_add(x[:, sl], x[:, sl], lnb_t[:, sl])
                oeng = nc.sync if ch % 2 == 0 else nc.gpsimd
                oeng.dma_start(
                    out=out_d.ap()[lb * 128:(lb + 1) * 128, sl], in_=x[:, sl])


def _pow2scale(w, target=120.0):
    m = float(np.abs(w).max())
    if m == 0.0:
        return 1.0
    return float(2.0 ** np.floor(np.log2(target / m)))


def _f8(x):
    return np.clip(x, -240.0, 240.0).astype(ml_dtypes.float8_e4m3)


def _host_prep(query, key, value, Wq, Wk, Wv, Wo, bo, ln_g, ln_b, alpha, beta):
    """Host-side: cumulative_state shortcut + fp8 layout/dtype marshaling."""
    a, b = float(alpha), float(beta)
    f64 = np.float64
    # mean over (batch, l) of kv[b,h,l,m] = (1/(B*L)) sum_b Ksum[b,h,:].V[b,h,m,:]
    keysum = key.astype(f64).sum(axis=1)                      # [B, D]
    Ksum = (keysum @ Wk.T.astype(f64)).reshape(B, H, DH)      # [B, H, DH]
    WvH = Wv.astype(f64).reshape(H, DH, D)
    wv_eff = np.einsum("hdj,bhd->bhj", WvH, Ksum, optimize=True)      # [B,H,D]
    contrib = np.einsum("bmj,bhj->hm", value.astype(f64), wv_eff, optimize=True)
    mean_kv = contrib / (B * L)                               # [H, DH]
    cs = np.zeros((H, DH), f64)
    c = np.zeros(DH, f64)
    for h in range(H):
        cs[h] = c
        c = b * c + a * mean_kv[h]
    # q_mod = Q*((1-a)*cs + a*kv) = (a*Q)*(kv + (1-a)/a*cs); a is folded
    # into Wq below, and this is cs*(1-a)/a:
    csp = ((1.0 - a) / a * cs if a != 0 else 0.0 * cs).astype(np.float32)
    csp_dev = np.ascontiguousarray(
        csp.reshape(H, 2, 128).transpose(2, 0, 1).reshape(128, H * 2)
    )
    plain_ln = bool(np.all(ln_g == 1.0) and np.all(ln_b == 0.0))

    bf = ml_dtypes.bfloat16
    wq_eff = a * Wq.T
    wk_eff, wv_eff_m, wo_eff = Wk.T, Wv.T, Wo.T
    scales = {
        "q": _pow2scale(wq_eff), "k": _pow2scale(wk_eff),
        "v": _pow2scale(wv_eff_m), "o": _pow2scale(wo_eff),
    }

    qT = _f8(np.ascontiguousarray(query.transpose(0, 2, 1)))
    kT = _f8(np.ascontiguousarray(key.transpose(0, 2, 1)))
    vT = _f8(np.ascontiguousarray(value.transpose(0, 2, 1)))
    wqT = _f8(np.ascontiguousarray(wq_eff * scales["q"]))
    wkT = _f8(np.ascontiguousarray(wk_eff * scales["k"]))
    wvT = _f8(np.ascontiguousarray(wv_eff_m * scales["v"]))
    woT = _f8(np.ascontiguousarray(wo_eff * scales["o"]))
    qres = (query + bo[None, None, :]).astype(bf)
    maskT = _f8(np.triu(np.ones((L, L), np.float32)))  # maskT[i,l]=1 iff i<=l

    in_maps = []
    for c_ in range(B):
        in_maps.append({
            "qT": qT[c_], "kT": kT[c_], "vT": vT[c_],
            "qres": qres[c_],
            "wqT": wqT, "wkT": wkT, "wvT": wvT, "woT": woT,
            "csp": csp_dev, "maskT": maskT,
            "lng": ln_g.astype(np.float32), "lnb": ln_b.astype(np.float32),
        })
    return in_maps, scales, plain_ln


def get_nc(scales, plain_ln: bool = True):
    key = (tuple(sorted(scales.items())), bool(plain_ln))
    if key not in _cache:
        _cache[key] = _build(scales, bool(plain_ln))
    return _cache[key]


def kernel(query, key, value, Wq, Wk, Wv, Wo, bo, ln_g, ln_b, alpha, beta,
           _trace=False, _trace_kwargs=None):
    args = [np.asarray(x) for x in
            (query, key, value, Wq, Wk, Wv, Wo, bo, ln_g, ln_b, alpha, beta)]
    in_maps, scales, plain_ln = _host_prep(*args)
    nc = get_nc(scales, plain_ln)
    res = run_bass_kernel_spmd(
        nc, in_maps, core_ids=list(range(B)),
        trace=_trace, **(_trace_kwargs or {}),
    )
    out = np.stack([res.results[c]["out"] for c in range(B)], axis=0)
    if _trace:
        kernel._last_results = res
    return out


# revision 11
# speedup vs baseline: 1.4899x; 1.0252x over previous
"""DeltaNet attention TRN2 kernel (nn_DeltaNetAttention_5299989643476).

Strategy: data-parallel over batch (8 batches -> 8 NeuronCores). The
cross-batch cumulative_state scan is tiny ([H, Dh]) and is computed on the
host via an algebraic shortcut, then passed to every core as a small
constant tensor, so the device program needs no collectives.

All four projections run as fp8e4m3 DoubleRow matmuls (2x PE throughput,
half the weight DMA). The input projections are "flipped": the fp8
activation tile (feature-major, already in SBUF) is the stationary operand
and the weight panel streams as the 512-wide moving operand, so the matmul
dominates LDWEIGHTS. All weights/inputs are pre-arranged on the host into
the exact SBUF layout so every DMA is one contiguous segment per partition
(128 descriptors instead of 2048 -> tiny issue cost). The projections
produce token-major K/V/Q; feature-major copies (KT/VT/QT and phi(K))
are made by the DMA xbar transpose on the sync queue, ring-ordered after
all weight transfers so nothing stalls behind them. phi(x)=elu(x)+1 is
relu(x) + min(exp(x), 1): exp/relu on ScalarE, fused min+add on DVE;
phi(K) runs early on the token-major projection output. The
cumulative-state offset is accumulated into the kv PSUM by a rank-1
matmul so q-mod is a plain tensor-tensor multiply. attnT is written as
fp8 so the output projection also runs DoubleRow; it iterates
seq-block-outer so block 0's LayerNorm (all-DVE, one ScalarE Rsqrt)
overlaps block 1's matmuls.
"""

import numpy as np
import ml_dtypes

import concourse.bass as bass
import concourse.mybir as mybir
import concourse.tile as tile
from concourse import bacc
from concourse.bass_utils import run_bass_kernel_spmd


def _ensure_axon_hooks():
    """This image's `antenv` lacks `axon_hooks`; if the caller's environment
    sets BASS_TRACE, run_bass_kernel_spmd would crash importing it. Register
    a no-op shim (only when absent) so tracing degrades gracefully."""
    try:
        import antenv.axon_hooks  # noqa: F401
    except ImportError:
        import sys
        import types

        import antenv

        mod = types.ModuleType("antenv.axon_hooks")
        _h = [None]
        mod.set_axon_ntff_profile_hook = lambda h: _h.__setitem__(0, h)
        mod.get_axon_ntff_profile_hook = lambda: _h[0]
        sys.modules["antenv.axon_hooks"] = mod
        antenv.axon_hooks = mod


_ensure_axon_hooks()

B, L, D, H = 8, 256, 2048, 8
DH = D // H            # 256
NB = D // 128          # 16 feature blocks of 128
LB = L // 128          # 2 sequence blocks of 128
EPS = 1e-5

F32 = mybir.dt.float32
BF16 = mybir.dt.bfloat16
F8 = mybir.dt.float8e4
AF = mybir.ActivationFunctionType
OP = mybir.AluOpType
DR = mybir.MatmulPerfMode.DoubleRow

_cache = {}


def _build(scales, plain_ln: bool = False):
    nc = bacc.Bacc(
        "TRN2",
        target_bir_lowering=False,
        debug=False,
        enable_asserts=False,
        num_devices=B,
    )

    # all big tensors pre-arranged on the host into SBUF layout:
    # per-partition data is one contiguous segment
    qT_d = nc.dram_tensor("qT", [128, NB, L], F8, kind="ExternalInput")
    kT_d = nc.dram_tensor("kT", [128, NB, L], F8, kind="ExternalInput")
    vT_d = nc.dram_tensor("vT", [128, NB, L], F8, kind="ExternalInput")
    qres_d = nc.dram_tensor("qres", [L, D], BF16, kind="ExternalInput")
    wq_d = nc.dram_tensor("wq", [128, 4, NB, 512], F8, kind="ExternalInput")
    wk_d = nc.dram_tensor("wk", [128, 4, NB, 512], F8, kind="ExternalInput")
    wv_d = nc.dram_tensor("wv", [128, 4, NB, 512], F8, kind="ExternalInput")
    wo_d = nc.dram_tensor("wo", [128, NB, D], F8, kind="ExternalInput")
    csp_d = nc.dram_tensor("csp", [1, D], BF16, kind="ExternalInput")
    maskT_d = nc.dram_tensor("maskT", [128, LB, L], F8, kind="ExternalInput")
    lng_d = nc.dram_tensor("lng", [D], F32, kind="ExternalInput")
    lnb_d = nc.dram_tensor("lnb", [D], F32, kind="ExternalInput")
    out_d = nc.dram_tensor("out", [L, D], F32, kind="ExternalOutput")

    with tile.TileContext(nc) as tc:
        _body(
            tc, scales,
            qT_d, kT_d, vT_d, qres_d,
            wq_d, wk_d, wv_d, wo_d,
            csp_d, maskT_d, lng_d, lnb_d, out_d,
            plain_ln,
        )
    nc.compile()
    return nc


def _body(tc, scales, qT_d, kT_d, vT_d, qres_d, wq_d, wk_d, wv_d, wo_d,
          csp_d, maskT_d, lng_d, lnb_d, out_d, plain_ln):
    nc = tc.nc
    inv_s = {k: float(1.0 / v) for k, v in scales.items()}

    with (
        tc.tile_pool(name="singles", bufs=1) as singles,
        tc.tile_pool(name="wpool", bufs=3) as wpool,
        tc.tile_pool(name="big", bufs=1) as big,
        tc.tile_pool(name="hgrp", bufs=2) as hgrp,
        tc.tile_pool(name="small", bufs=3) as small,
        tc.tile_pool(name="psA", bufs=2, space="PSUM") as psA,
        tc.tile_pool(name="an_ps", bufs=2, space="PSUM") as an_ps,
        tc.tile_pool(name="dv_ps", bufs=2, space="PSUM") as dv_ps,
    ):
        # ---- input activations (feature-major fp8, used as DR stationary) ----
        xT_in = {}
        for name, dram in (("k", kT_d), ("v", vT_d), ("q", qT_d)):
            t = big.tile([128, NB, L], F8, tag=f"{name}T_in", name=f"{name}T_in")
            xT_in[name] = (t, dram)

        def load_xT(name, h2_eng=None):
            t, dram = xT_in[name]
            r = dram.ap()
            nc.gpsimd.dma_start(out=t[:, 0:8, :], in_=r[:, 0:8, :])
            (h2_eng or nc.gpsimd).dma_start(out=t[:, 8:16, :], in_=r[:, 8:16, :])

        # critical path first: kT + the K weight panels
        load_xT("k", h2_eng=nc.scalar)

        w_ds = {"k": wk_d, "v": wv_d, "q": wq_d}

        def panel_dmas(name):
            ts = []
            for ic in range(4):
                w_t = wpool.tile([128, NB, 512], F8, tag="w",
                                 name=f"w_{name}{ic}")
                eng = nc.sync if ic % 2 == 0 else nc.scalar
                eng.dma_start(out=w_t, in_=w_ds[name].ap()[:, ic])
                ts.append(w_t)
            return ts

        wk_ts = panel_dmas("k")

        ones_t = singles.tile([128, 256], BF16)
        nc.vector.memset(ones_t, 1.0)
        eps_t = singles.tile([128, 1], F32)
        nc.vector.memset(eps_t, EPS)

        # dummy matmuls while the first panels stream: keeps the PE-HAM
        # activity monitor busy so the real stream starts at 2.4 GHz
        warm_ps = dv_ps.tile([128, 256], F32, tag="dv", name="warm_ps")
        for _ in range(24):
            nc.tensor.matmul(warm_ps[:, 0:128], ones_t[:, 0:128],
                             ones_t[:, 0:128], start=True, stop=True)

        csp_t = singles.tile([1, D], BF16)
        nc.gpsimd.dma_start(out=csp_t, in_=csp_d.ap())

        # ---- flipped projections: X[l, i] = sum_j xT[j, l] * wT[j, i] ----
        K_tok = big.tile([128, LB, D], BF16, tag="ktok", name="ktok")
        V_t = big.tile([128, LB, D], BF16, tag="V", name="V")
        Q_tok = big.tile([128, LB, D], BF16, tag="qtok", name="qtok")

        def flip_proj(name, tok_t, w_ts):
            x_t = xT_in[name][0]
            for ic in range(4):
                w_t = w_ts[ic]
                isl = slice(ic * 512, (ic + 1) * 512)
                ps = psA.tile([128, 2, 512], F32, tag="pk")
                for lb in range(LB):
                    for jj in range(8):
                        nc.tensor.matmul(
                            ps[:, lb, :],
                            x_t[:, 2 * jj:2 * jj + 2, lb * 128:(lb + 1) * 128],
                            w_t[:, 2 * jj:2 * jj + 2, :],
                            start=(jj == 0),
                            stop=(jj == 7),
                            perf_mode=DR,
                        )
                nc.vector.tensor_scalar(
                    out=tok_t[:, :, isl], in0=ps,
                    scalar1=inv_s[name], scalar2=None, op0=OP.mult,
                )

        # feature-major [128, lb, n, l2] tiles via DMA xbar transpose; each
        # lb slice is a contiguous destination (required for correctness).
        # All transposes ride the sync queue, ring-ordered after the weights.
        def xbar_T(tok_t, nm):
            t = big.tile([128, LB, NB, 128], BF16, tag=f"{nm}T2",
                         name=f"{nm}T2")
            for lb in range(LB):
                nc.sync.dma_start_transpose(out=t[:, lb], in_=tok_t[:, lb, :])
            return t

        flip_proj("k", K_tok, wk_ts)
        wv_ts = panel_dmas("v")
        load_xT("v")

        # pk_tok = phi(K_tok) = relu + min(exp, 1), chunked right behind the
        # K projection copies (ScalarE exp/relu + DVE fused min-add)
        pk_tok = big.tile([128, LB, D], BF16, tag="pktok", name="pktok")
        for ch in range(4):
            sl = slice(ch * 512, (ch + 1) * 512)
            ex = hgrp.tile([128, LB, 512], BF16, tag="eq")
            rx = hgrp.tile([128, LB, 512], BF16, tag="rx")
            nc.scalar.activation(ex, K_tok[:, :, sl], AF.Exp)
            nc.scalar.activation(rx, K_tok[:, :, sl], AF.Relu)
            nc.vector.scalar_tensor_tensor(
                out=pk_tok[:, :, sl], in0=ex, scalar=1.0, in1=rx,
                op0=OP.min, op1=OP.add,
            )

        flip_proj("v", V_t, wv_ts)
        wq_ts = panel_dmas("q")
        load_xT("q")

        # wo preloaded whole; single contiguous DMA behind the q panels
        wo_t = singles.tile([128, NB, D], F8, name="wo")
        nc.sync.dma_start(out=wo_t, in_=wo_d.ap())

        KT = xbar_T(K_tok, "k")
        pkT = xbar_T(pk_tok, "pk")
        VT = xbar_T(V_t, "v")

        maskT_t = singles.tile([128, LB, L], F8)
        nc.gpsimd.dma_start(out=maskT_t, in_=maskT_d.ap())
        qres_t = []
        for lb in range(LB):
            t = big.tile([128, D], BF16, tag=f"qres{lb}", name=f"qres{lb}")
            nc.gpsimd.dma_start(out=t, in_=qres_d.ap()[lb * 128:(lb + 1) * 128, :])
            qres_t.append(t)
        lng_t = lnb_t = None
        if not plain_ln:
            lng_t = singles.tile([128, D], F32)
            nc.gpsimd.dma_start(out=lng_t,
                                in_=lng_d.ap().partition_broadcast(128))
            lnb_t = singles.tile([128, D], F32)
            nc.gpsimd.dma_start(out=lnb_t,
                                in_=lnb_d.ap().partition_broadcast(128))

        flip_proj("q", Q_tok, wq_ts)
        QT = xbar_T(Q_tok, "q")

        # ---- attention: per head-group kv (+cs via rank-1 matmul) + phi ----
        pq_t = big.tile([128, NB, L], BF16, tag="pq_t", name="pq_t")
        attnT_t = big.tile([128, NB, L], F8, tag="attnT", name="attnT")

        for g in range(4):
            gsl = slice(4 * g, 4 * g + 4)
            kvm = hgrp.tile([128, 4, L], BF16, tag="kvm")
            for hh in range(2):
                h = 2 * g + hh
                n0 = 2 * h
                ps = psA.tile([128, 2, L], F32, tag="pk")
                for mb in range(2):
                    for db in range(2):
                        for lb in range(LB):
                            nc.tensor.matmul(
                                ps[:, mb, lb * 128:(lb + 1) * 128],
                                VT[:, mb, n0 + db, :],
                                KT[:, lb, n0 + db, :],
                                start=(db == 0 and lb == 0),
                                stop=False,
                            )
                    # += cs[m] (outer product with ones): kv + cs in PSUM
                    nc.tensor.matmul(
                        ps[:, mb, :],
                        csp_t[0:1, (n0 + mb) * 128:(n0 + mb + 1) * 128],
                        ones_t[0:1, :],
                        start=False,
                        stop=True,
                    )
                for mb in range(2):
                    # q_mod = (alpha*Q) * (kv + cs'); alpha folded into Wq
                    nc.vector.tensor_mul(
                        kvm[:, 2 * hh + mb, :],
                        ps[:, mb, :],
                        QT[:, :, n0 + mb, :],
                    )
            # pq = phi(kvm)
            eq = hgrp.tile([128, 4, L], BF16, tag="eqw")
            rq = hgrp.tile([128, 4, L], BF16, tag="rqw")
            nc.scalar.activation(eq, kvm, AF.Exp)
            nc.scalar.activation(rq, kvm, AF.Relu)
            nc.vector.scalar_tensor_tensor(
                out=pq_t[:, gsl, :], in0=eq, scalar=1.0, in1=rq,
                op0=OP.min, op1=OP.add,
            )

        # ---- per-head: A matmul, mask, den, num, attnT ----
        for h in range(H):
            n0 = 2 * h
            # causal block structure of AT[i, l] (i<=l kept):
            #   ib=0: l<128 lower-triangular, l>=128 all-ones
            #   ib=1: l<128 all-zero (skipped entirely), l>=128 triangular
            a_ps = an_ps.tile([128, 2, L], F32, tag="an")
            for db in range(2):
                nc.tensor.matmul(
                    a_ps[:, 0, :],
                    pkT[:, 0, n0 + db, :],
                    pq_t[:, n0 + db, :],
                    start=(db == 0), stop=(db == 1),
                )
            for db in range(2):
                nc.tensor.matmul(
                    a_ps[:, 1, 128:L],
                    pkT[:, 1, n0 + db, :],
                    pq_t[:, n0 + db, 128:L],
                    start=(db == 0), stop=(db == 1),
                )
            am = small.tile([128, LB, L], BF16, tag="am")
            nc.vector.tensor_mul(am[:, 0, 0:128], a_ps[:, 0, 0:128],
                                 maskT_t[:, 0, 0:128])
            nc.scalar.copy(out=am[:, 0, 128:L], in_=a_ps[:, 0, 128:L])
            nc.vector.tensor_mul(am[:, 1, 128:L], a_ps[:, 1, 128:L],
                                 maskT_t[:, 1, 128:L])

            den_ps = dv_ps.tile([128, L], F32, tag="dv", name="den_ps")
            nc.tensor.matmul(den_ps[:, 0:128], ones_t[:, 0:128],
                             am[:, 0, 0:128], start=True, stop=True)
            nc.tensor.matmul(den_ps[:, 128:L], ones_t[:, 0:128],
                             am[:, 0, 128:L], start=True, stop=False)
            nc.tensor.matmul(den_ps[:, 128:L], ones_t[:, 0:128],
                             am[:, 1, 128:L], start=False, stop=True)
            # den is a sum of strictly positive phi-products, so the
            # reference's 1e-8 clamp can never bind.
            rden = small.tile([128, L], F32, tag="rden")
            nc.vector.reciprocal_approx_fast(out=rden, in_=den_ps)

            n_ps = an_ps.tile([128, 2, L], F32, tag="an")
            for db in range(2):
                v0 = V_t[:, 0, h * DH + db * 128:h * DH + (db + 1) * 128]
                v1 = V_t[:, 1, h * DH + db * 128:h * DH + (db + 1) * 128]
                nc.tensor.matmul(n_ps[:, db, 0:128], v0, am[:, 0, 0:128],
                                 start=True, stop=True)
                nc.tensor.matmul(n_ps[:, db, 128:L], v0, am[:, 0, 128:L],
                                 start=True, stop=False)
                nc.tensor.matmul(n_ps[:, db, 128:L], v1, am[:, 1, 128:L],
                                 start=False, stop=True)
            for db in range(2):
                nc.vector.tensor_mul(attnT_t[:, n0 + db, :], n_ps[:, db, :], rden)

        # Preload the Rsqrt ACT table now — after ScalarE's last Copy/Exp
        # user, off the LN tail's critical path.
        warm_rs = singles.tile([128, 1], F32)
        nc.scalar.activation(warm_rs, eps_t, AF.Sqrt)

        # ---- output projection (fp8 DR, stationary=attnT) + residual + LN ----
        # seq-block-outer so lb=0's LN + output DMA overlap lb=1's matmuls
        x_sb = [big.tile([128, D], F32, tag=f"x{lb}", name=f"x{lb}")
                for lb in range(LB)]
        stats = [small.tile([128, 4, 6], F32, tag=f"stats{lb}",
                            name=f"stats{lb}", bufs=1) for lb in range(LB)]
        for lb in range(LB):
            for nq in range(4):
                psf = psA.tile([128, 512], F32, tag="pk")
                for jj in range(8):
                    nc.tensor.matmul(
                        psf,
                        attnT_t[:, 2 * jj:2 * jj + 2, lb * 128:(lb + 1) * 128],
                        wo_t[:, 2 * jj:2 * jj + 2, nq * 512:(nq + 1) * 512],
                        start=(jj == 0),
                        stop=(jj == 7),
                        perf_mode=DR,
                    )
                # x = o/s_o + (query + bo)
                sl = slice(nq * 512, (nq + 1) * 512)
                nc.vector.scalar_tensor_tensor(
                    out=x_sb[lb][:, sl], in0=psf, scalar=inv_s["o"],
                    in1=qres_t[lb][:, sl], op0=OP.mult, op1=OP.add,
                )
                nc.vector.bn_stats(out=stats[lb][:, nq, :],
                                   in_=x_sb[lb][:, sl])

            x = x_sb[lb]
            mv = small.tile([128, 2], F32, tag="mv")
            nc.vector.bn_aggr(out=mv, in_=stats[lb])
            sd = small.tile([128, 1], F32, tag="sd")
            nc.scalar.activation(sd, mv[:, 1:2], AF.Sqrt, bias=eps_t)
            nc.vector.reciprocal_approx_fast(out=sd, in_=sd)
            for ch in range(2):  # halves, so DVE work overlaps output DMA
                sl = slice(ch * (D // 2), (ch + 1) * (D // 2))
                if plain_ln:
                    # ln_g == 1, ln_b == 0: fused (x - mu) * rstd on DVE
                    nc.vector.tensor_scalar(
                        out=x[:, sl], in0=x[:, sl], scalar1=mv[:, 0:1],
                        scalar2=sd, op0=OP.subtract, op1=OP.mult,
                    )
                else:
                    nc.vector.tensor_scalar(
                        out=x[:, sl], in0=x[:, sl], scalar1=mv[:, 0:1],
                        scalar2=sd, op0=OP.subtract, op1=OP.mult,
                    )
                    nc.vector.tensor_mul(x[:, sl], x[:, sl], lng_t[:, sl])
                    nc.vector.tensor_add(x[:, sl], x[:, sl], lnb_t[:, sl])
                oeng = nc.sync if ch % 2 == 0 else nc.gpsimd
                oeng.dma_start(
                    out=out_d.ap()[lb * 128:(lb + 1) * 128, sl], in_=x[:, sl])


def _pow2scale(w, target=120.0):
    m = float(np.abs(w).max())
    if m == 0.0:
        return 1.0
    return float(2.0 ** np.floor(np.log2(target / m)))


def _f8(x):
    return np.clip(x, -240.0, 240.0).astype(ml_dtypes.float8_e4m3)


def _sbufize_w(wT):
    """[D, D] -> [128, 4, NB, 512]: panel (ic) of SBUF row p is contiguous."""
    # w[p, ic, n, i2] = wT[n*128+p, ic*512+i2]
    w = wT.reshape(NB, 128, 4, 512)          # [n, p, ic, i2]
    return np.ascontiguousarray(w.transpose(1, 2, 0, 3))


def _sbufize_x(xT):
    """[D, L] -> [128, NB, L]: SBUF row p is contiguous."""
    x = xT.reshape(NB, 128, L)
    return np.ascontiguousarray(x.transpose(1, 0, 2))


def _host_prep(query, key, value, Wq, Wk, Wv, Wo, bo, ln_g, ln_b, alpha, beta):
    """Host-side: cumulative_state shortcut + fp8 layout/dtype marshaling."""
    a, b = float(alpha), float(beta)
    f64 = np.float64
    # mean over (batch, l) of kv[b,h,l,m] = (1/(B*L)) sum_b Ksum[b,h,:].V[b,h,m,:]
    keysum = key.astype(f64).sum(axis=1)                      # [B, D]
    Ksum = (keysum @ Wk.T.astype(f64)).reshape(B, H, DH)      # [B, H, DH]
    WvH = Wv.astype(f64).reshape(H, DH, D)
    wv_eff = np.einsum("hdj,bhd->bhj", WvH, Ksum, optimize=True)      # [B,H,D]
    contrib = np.einsum("bmj,bhj->hm", value.astype(f64), wv_eff, optimize=True)
    mean_kv = contrib / (B * L)                               # [H, DH]
    cs = np.zeros((H, DH), f64)
    c = np.zeros(DH, f64)
    for h in range(H):
        cs[h] = c
        c = b * c + a * mean_kv[h]
    # q_mod = Q*((1-a)*cs + a*kv) = (a*Q)*(kv + (1-a)/a*cs); a is folded
    # into Wq below, and this is cs*(1-a)/a:
    csp = ((1.0 - a) / a * cs if a != 0 else 0.0 * cs).astype(np.float32)
    plain_ln = bool(np.all(ln_g == 1.0) and np.all(ln_b == 0.0))

    bf = ml_dtypes.bfloat16
    wq_eff = a * Wq.T
    wk_eff, wv_eff_m, wo_eff = Wk.T, Wv.T, Wo.T
    scales = {
        "q": _pow2scale(wq_eff), "k": _pow2scale(wk_eff),
        "v": _pow2scale(wv_eff_m), "o": _pow2scale(wo_eff),
    }

    qT = np.stack([_sbufize_x(_f8(query[c_].T)) for c_ in range(B)])
    kT = np.stack([_sbufize_x(_f8(key[c_].T)) for c_ in range(B)])
    vT = np.stack([_sbufize_x(_f8(value[c_].T)) for c_ in range(B)])
    wq8 = _sbufize_w(_f8(wq_eff * scales["q"]))
    wk8 = _sbufize_w(_f8(wk_eff * scales["k"]))
    wv8 = _sbufize_w(_f8(wv_eff_m * scales["v"]))
    # wo keeps [128, NB, D]: w[p, n, i] = woT[n*128+p, i]
    wo8 = np.ascontiguousarray(
        _f8(wo_eff * scales["o"]).reshape(NB, 128, D).transpose(1, 0, 2))
    qres = (query + bo[None, None, :]).astype(bf)
    csp_row = np.ascontiguousarray(csp.reshape(1, D)).astype(bf)
    mask = np.triu(np.ones((L, L), np.float32))   # mask[i, l] = 1 iff i <= l
    maskT = _f8(np.ascontiguousarray(
        mask.reshape(LB, 128, L).transpose(1, 0, 2)))

    in_maps = []
    for c_ in range(B):
        in_maps.append({
            "qT": qT[c_], "kT": kT[c_], "vT": vT[c_],
            "qres": qres[c_],
            "wq": wq8, "wk": wk8, "wv": wv8, "wo": wo8,
            "csp": csp_row, "maskT": maskT,
            "lng": ln_g.astype(np.float32), "lnb": ln_b.astype(np.float32),
        })
    return in_maps, scales, plain_ln


def get_nc(scales, plain_ln: bool = True):
    key = (tuple(sorted(scales.items())), bool(plain_ln))
    if key not in _cache:
        _cache[key] = _build(scales, bool(plain_ln))
    return _cache[key]


def kernel(query, key, value, Wq, Wk, Wv, Wo, bo, ln_g, ln_b, alpha, beta,
           _trace=False, _trace_kwargs=None):
    args = [np.asarray(x) for x in
            (query, key, value, Wq, Wk, Wv, Wo, bo, ln_g, ln_b, alpha, beta)]
    in_maps, scales, plain_ln = _host_prep(*args)
    nc = get_nc(scales, plain_ln)
    res = run_bass_kernel_spmd(
        nc, in_maps, core_ids=list(range(B)),
        trace=_trace, **(_trace_kwargs or {}),
    )
    out = np.stack([res.results[c]["out"] for c in range(B)], axis=0)
    if _trace:
        kernel._last_results = res
    return out
